# revision 10
# baseline (speedup 1.0000x reference)
"""CrossAttnBlock kernel for 8 Trainium2 NeuronCores.

Sharding: data-parallel over the batch dim B=8 -> one batch item per core.
Each core runs the full block (q/kv projections, cross-attention, merge,
FFN) on its [1024, 512] slice; weights are replicated.

v2: fp8 (e4m3) DoubleRow matmuls for all dense GEMMs -- each matmul
instruction contracts 2x128 K-rows (weights host-scaled by 256 to sit in
fp8 normal range; the 1/256 is folded into the PSUM readout scale).
Attention S^T stays bf16 (K=64) but head PAIRS are issued back-to-back so
even/odd heads run concurrently on PE row-groups 0-63 / 64-127.
The per-head v bias is folded host-side into the residual seed
(attn rows sum to 1 after normalization, so attn@(v+b)@W = attn@v@W + b@W).

Layout (per core): activations transposed so every matmul contracts over
the partition dim:
  xaT/ctxaT [d, n] fp8   (LN+swish fused: silu((x-mu)*rs) via act scale/bias)
  qT/kT     [c, n] bf16  (feed S only)
  vv_h      [m, c] fp8   (scaled 8x: readout 256*v -> *1/32)
  S^T[m, n] -> expS fp8 = exp(S*scale)
  den       = onesT @ expS (fp8 DR), *64 at readout; recip includes /64
  outT[c,n] fp8 = (v.T @ expS)/32  (so merge PSUM = 64*delta_unnorm)
  delta[n,c] += (outT.T @ mw) * recip        (mul on DVE, add on GpSimd)
  FFN: LN/swish -> transpose -> h1T (silu(ps/256+b1)) -> ff + residual
"""

import json

import numpy as np

import concourse.bass as bass
import concourse.mybir as mybir
import concourse.tile as tile
from concourse.bass_utils import run_bass_kernel_spmd

F32 = mybir.dt.float32
BF16 = mybir.dt.bfloat16
F8 = mybir.dt.float8e4
AF = mybir.ActivationFunctionType
DR = mybir.MatmulPerfMode.DoubleRow

P = 128
N = 1024          # query rows per core
M = 1024          # context rows per core
D = 512           # d_in == d_ctx == d_out
H = 8             # heads
DH = 64           # head dim (k/q)
DE = 2048         # ffn expand
KC = D // P       # 4 feature chunks
NCH = N // P      # 8 row chunks
ECH = DE // P     # 16 expand chunks
SCALE = DH ** -0.5
EPS = 1e-5
NS = 2            # free-dim split of 1024 into 2x512
FD = 512          # matmul moving free dim
WS = 256.0        # host-side fp8 weight scale


# --- workaround: this walrus build allows only ONE embedded sync wait per
# instruction. Hoist all but the last wait of every instruction onto
# preceding single-wait NoOps on the same engine.

def _split_multiwait_drains(bir_json: bytes) -> bytes:
    d = json.loads(bir_json)
    changed = False
    for fn in d.get("functions", []):
        for blk in fn.get("blocks", []):
            out = []
            for inst in blk.get("instructions", []):
                si = inst.get("sync_info") or {}
                waits = si.get("on_wait") or []
                if len(waits) > 1:
                    for j, w in enumerate(waits[:-1]):
                        out.append({
                            "name": f"{inst['name']}__w{j}",
                            "engine": inst["engine"],
                            "opcode": "NoOp",
                            "ins": [],
                            "outs": [],
                            "debug": inst.get("debug"),
                            "sync_info": {"on_wait": [w], "on_update": []},
                        })
                    si["on_wait"] = [waits[-1]]
                    changed = True
                out.append(inst)
            blk["instructions"] = out
    if not changed:
        return bir_json
    return json.dumps(d).encode()


def _install_compat():
    import concourse.bass_utils as bu
    import concourse.bass2jax as b2j

    if getattr(b2j, "_drain_split_installed", False):
        return
    orig = bu.compile_bir_kernel

    def patched(bir_json, tmpdir, neff_name="file.neff"):
        return orig(_split_multiwait_drains(bir_json), tmpdir, neff_name)

    b2j.compile_bir_kernel = patched
    b2j._drain_split_installed = True


def _bcast_1d(t, n):
    """DRAM [n] vector -> AP broadcast to [P, n] (partition stride 0)."""
    ap = t.ap()
    return bass.AP(tensor=ap.tensor, offset=ap.offset, ap=[[0, P], ap.ap[0]])


def _build(skip_gb=False):
    nc = bass.Bass("TRN2")

    x_d = nc.dram_tensor("x", [N, D], F32, kind="ExternalInput")
    ctx_d = nc.dram_tensor("context", [M, D], F32, kind="ExternalInput")
    qg_d = nc.dram_tensor("q_g", [D], F32, kind="ExternalInput")
    qb_d = nc.dram_tensor("q_b", [D], F32, kind="ExternalInput")
    qw_d = nc.dram_tensor("q_w", [D, DH * H], F8, kind="ExternalInput")
    qbias_d = nc.dram_tensor("q_bias", [DH * H], F32, kind="ExternalInput")
    kvg_d = nc.dram_tensor("kv_g", [D], F32, kind="ExternalInput")
    kvb_d = nc.dram_tensor("kv_b", [D], F32, kind="ExternalInput")
    kvw_d = nc.dram_tensor("kv_w", [D, (DH + D) * H], F8, kind="ExternalInput")
    kvbias_d = nc.dram_tensor("kv_bias", [(DH + D) * H], F32, kind="ExternalInput")
    mw_d = nc.dram_tensor("merge_w", [D * H, D], F8, kind="ExternalInput")
    mbr_d = nc.dram_tensor("mbr", [D], F32, kind="ExternalInput")
    ffg_d = nc.dram_tensor("ff_g", [D], F32, kind="ExternalInput")
    ffb_d = nc.dram_tensor("ff_b", [D], F32, kind="ExternalInput")
    fw1_d = nc.dram_tensor("ff_w1", [D, DE], F8, kind="ExternalInput")
    fb1_d = nc.dram_tensor("ff_b1", [DE], F32, kind="ExternalInput")
    fw2_d = nc.dram_tensor("ff_w2", [DE, D], F8, kind="ExternalInput")
    fb2_d = nc.dram_tensor("ff_b2", [D], F32, kind="ExternalInput")
    out_d = nc.dram_tensor("out", [N, D], F32, kind="ExternalOutput")

    from concourse.masks import make_identity

    with tile.TileContext(nc) as tc:
        with (
            tc.tile_pool(name="persist", bufs=1) as pers,
            tc.tile_pool(name="resid", bufs=1) as resid_pool,
        ):
            ident = pers.tile([P, P], F32, tag="ident")
            make_identity(nc, ident)
            ident_bf = pers.tile([P, P], BF16, tag="ident_bf")
            nc.vector.tensor_copy(out=ident_bf, in_=ident)
            eps_t = pers.tile([P, 1], F32, tag="eps")
            nc.vector.memset(eps_t, EPS)
            ones_col = pers.tile([P, 1], F32, tag="ones")
            nc.vector.memset(ones_col, 1.0)
            # fp8 ones for the DoubleRow denominator matmul: lhsT must be a
            # 3D AP [128, 2, 1] whose middle-dim byte step is 16-aligned.
            ones8 = pers.tile([P, 2, 16], F8, tag="ones8")
            nc.vector.memset(ones8, 1.0)

            delta = [
                resid_pool.tile([P, D], F32, tag=f"delta{j}", name=f"delta{j}")
                for j in range(NCH)
            ]

            # per-partition-column biases
            with nc.allow_non_contiguous_dma(reason="tiny bias gathers"):
                qbias_c = pers.tile([P, KC], F32, tag="qbias")
                nc.gpsimd.dma_start(qbias_c, qbias_d.ap().rearrange("(o p) -> p o", p=P))
                kvbk_c = pers.tile([P, KC], F32, tag="kvbk")
                nc.gpsimd.dma_start(
                    kvbk_c, kvbias_d.ap()[0:DH * H].rearrange("(o p) -> p o", p=P)
                )
                fb1_c = pers.tile([P, ECH], F32, tag="fb1")
                nc.gpsimd.dma_start(fb1_c, fb1_d.ap().rearrange("(o p) -> p o", p=P))

            def ln_chunk(src_d, xt, g_b, b_b, pool, tag, out_t=None):
                """LN stats + fused silu((x-mu)*rs) for one 128-row chunk.
                Returns the bf16 silu output tile."""
                st = pool.tile([P, 6], F32, tag=f"{tag}_st", name="st")
                nc.vector.bn_stats(out=st, in_=xt)
                mv = pool.tile([P, 2], F32, tag=f"{tag}_mv", name="mv")
                nc.vector.bn_aggr(out=mv, in_=st)
                rs = pool.tile([P, 1], F32, tag=f"{tag}_rs", name="rs")
                nc.scalar.activation(
                    out=rs, in_=mv[:, 1:2], func=AF.Sqrt, bias=eps_t
                )
                nc.vector.reciprocal(out=rs, in_=rs)
                xab = out_t if out_t is not None else pool.tile(
                    [P, D], BF16, tag=f"{tag}_xab", name="xab")
                xa = pool.tile([P, D], F32, tag=f"{tag}_xa", name="xa")
                nc.vector.tensor_scalar(
                    out=xa, in0=xt, scalar1=mv[:, 0:1], scalar2=rs,
                    op0=mybir.AluOpType.subtract,
                    op1=mybir.AluOpType.mult,
                )
                if not skip_gb:
                    nc.gpsimd.tensor_mul(out=xa, in0=xa, in1=g_b)
                    nc.gpsimd.tensor_add(out=xa, in0=xa, in1=b_b)
                nc.scalar.activation(out=xab, in_=xa, func=AF.Silu)
                return xab

            def transpose_chunk(xab, dstT, j, psum_t):
                for kc in range(KC):
                    pt = psum_t.tile([P, P], BF16, tag="pt", name="pt")
                    nc.tensor.transpose(pt, xab[:, kc * P:(kc + 1) * P], ident_bf)
                    nc.vector.tensor_copy(
                        out=dstT[:, kc, j * P:(j + 1) * P], in_=pt
                    )

            def proj_T(w_sb, rhsT, dst, bias_c, pmm):
                """dst[c, n] = (w.T @ act)/WS + bias, bf16 (DoubleRow)."""
                for cc in range(KC):
                    for ns in range(NS):
                        ps = pmm.tile([P, FD], F32, tag="pmm", name="ps")
                        for g in range(KC // 2):
                            nc.tensor.matmul(
                                ps,
                                lhsT=w_sb[:, 2 * g:2 * g + 2, cc * P:(cc + 1) * P],
                                rhs=rhsT[:, 2 * g:2 * g + 2, ns * FD:(ns + 1) * FD],
                                start=(g == 0), stop=(g == KC // 2 - 1),
                                perf_mode=DR,
                            )
                        nc.vector.tensor_scalar(
                            out=dst[:, cc, ns * FD:(ns + 1) * FD],
                            in0=ps, scalar1=1.0 / WS, scalar2=bias_c[:, cc:cc + 1],
                            op0=mybir.AluOpType.mult, op1=mybir.AluOpType.add,
                        )

            # ---- weight tiles for phase C (allocated early, DMA'd last)
            phCw_cm = tc.tile_pool(name="phCw", bufs=1)
            tCw = phCw_cm.__enter__()
            fw1_sb = tCw.tile([P, KC, DE], F8, tag="fw1")
            fw2_sb = tCw.tile([P, ECH, D], F8, tag="fw2")
            fb2_b = tCw.tile([P, D], F32, tag="fb2")
            if not skip_gb:
                ffg_b = tCw.tile([P, D], F32, tag="ffg")
                ffb_b = tCw.tile([P, D], F32, tag="ffb")
            else:
                ffg_b = ffb_b = None

            # LN(x2) results, produced in phase B tail, used in phase C
            phBC_cm = tc.tile_pool(name="phBC", bufs=1)
            tBC = phBC_cm.__enter__()
            fabs = [tBC.tile([P, D], BF16, tag=f"fab{j}", name=f"fab{j}")
                    for j in range(NCH)]
            x2b = [tBC.tile([P, D], F32, tag=f"x2b{j}", name=f"x2b{j}")
                   for j in range(NCH)]

            # ---- activations + weights that span phases A+B
            acts_ab_cm = tc.tile_pool(name="actsAB", bufs=1)
            acts_ab = acts_ab_cm.__enter__()
            ctxaT = acts_ab.tile([P, KC, M], F8, tag="ctxaT")
            qT = acts_ab.tile([P, KC, N], BF16, tag="qT")
            kT = acts_ab.tile([P, KC, M], BF16, tag="kT")
            vv = [acts_ab.tile([P, NCH, D], F8, tag=f"vv{h}", name=f"vv{h}")
                  for h in range(H)]
            mw_sb = acts_ab.tile([P, KC * H, D], F8, tag="mw")

            # ---------------- phase A: LN/swish/transpose + projections + v
            with (
                tc.tile_pool(name="phA", bufs=3) as tA,
                tc.tile_pool(name="phA_in", bufs=1) as tAin,
                tc.tile_pool(name="phA_w", bufs=1) as tAw,
                tc.tile_pool(name="pmmA", bufs=4, space="PSUM") as pmm,
                tc.tile_pool(name="ptA", bufs=2, space="PSUM") as ptp,
            ):
                xaT = tAw.tile([P, KC, N], F8, tag="xaT")
                kvwv_sb = tAw.tile([P, KC, D * H], F8, tag="kvwv")
                qw_sb = tAw.tile([P, KC, DH * H], F8, tag="qw")
                kvwk_sb = tAw.tile([P, KC, DH * H], F8, tag="kvwk")
                mbr_b = tAw.tile([P, D], F32, tag="mbr")
                if not skip_gb:
                    qg_b = tAw.tile([P, D], F32, tag="qg")
                    nc.gpsimd.dma_start(qg_b, _bcast_1d(qg_d, D))
                    qb_b = tAw.tile([P, D], F32, tag="qb")
                    nc.gpsimd.dma_start(qb_b, _bcast_1d(qb_d, D))
                    kvg_b = tAw.tile([P, D], F32, tag="kvg")
                    nc.gpsimd.dma_start(kvg_b, _bcast_1d(kvg_d, D))
                    kvb_b = tAw.tile([P, D], F32, tag="kvb")
                    nc.gpsimd.dma_start(kvb_b, _bcast_1d(kvb_d, D))
                    nc.gpsimd.dma_start(ffg_b, _bcast_1d(ffg_d, D))
                    nc.gpsimd.dma_start(ffb_b, _bcast_1d(ffb_d, D))
                else:
                    qg_b = qb_b = kvg_b = kvb_b = None
                nc.gpsimd.dma_start(mbr_b, _bcast_1d(mbr_d, D))
                nc.gpsimd.dma_start(fb2_b, _bcast_1d(fb2_d, D))

                # one DMA queue, priority order: ctx, k/q weights, v weights,
                # x, merge weights, ffn weights -- so nothing starves ctx.
                ctx_t = []
                for j in range(NCH):
                    xt = tAin.tile([P, D], F32, tag=f"lc_in{j}", name="xt")
                    nc.sync.dma_start(xt, ctx_d.ap()[j * P:(j + 1) * P, :])
                    ctx_t.append(xt)
                nc.sync.dma_start(
                    kvwk_sb,
                    kvw_d.ap()[:, 0:DH * H].rearrange("(o p) c -> p o c", p=P),
                )
                nc.sync.dma_start(qw_sb, qw_d.ap().rearrange("(o p) c -> p o c", p=P))
                nc.sync.dma_start(
                    kvwv_sb,
                    kvw_d.ap()[:, DH * H:].rearrange("(o p) c -> p o c", p=P),
                )
                x_t = []
                for j in range(NCH):
                    xt = tAin.tile([P, D], F32, tag=f"lx_in{j}", name="xt2")
                    nc.sync.dma_start(xt, x_d.ap()[j * P:(j + 1) * P, :])
                    x_t.append(xt)
                nc.sync.dma_start(
                    mw_sb, mw_d.ap().rearrange("(o p) c -> p o c", p=P)
                )
                nc.sync.dma_start(
                    fw1_sb, fw1_d.ap().rearrange("(o p) c -> p o c", p=P)
                )
                nc.sync.dma_start(
                    fw2_sb, fw2_d.ap().rearrange("(o p) c -> p o c", p=P)
                )

                for j in range(NCH):
                    xab = ln_chunk(ctx_d, ctx_t[j], kvg_b, kvb_b, tA, "lc")
                    transpose_chunk(xab, ctxaT, j, ptp)
                proj_T(kvwk_sb, ctxaT, kT, kvbk_c, pmm)

                # v for all heads (scaled 8x), readouts split Scalar/DVE
                for h in range(H):
                    for i in range(NCH):
                        ps = pmm.tile([P, FD], F32, tag="pmm", name="ps")
                        for g in range(KC // 2):
                            nc.tensor.matmul(
                                ps,
                                lhsT=ctxaT[:, 2 * g:2 * g + 2, i * P:(i + 1) * P],
                                rhs=kvwv_sb[:, 2 * g:2 * g + 2,
                                            h * D:(h + 1) * D],
                                start=(g == 0), stop=(g == KC // 2 - 1),
                                perf_mode=DR,
                            )
                        if h % 2 == 0:
                            nc.scalar.activation(
                                out=vv[h][:, i, :], in_=ps, func=AF.Copy,
                                scale=8.0 / WS,
                            )
                        else:
                            nc.vector.tensor_scalar_mul(
                                out=vv[h][:, i, :], in0=ps, scalar1=8.0 / WS,
                            )

                # x LN (DVE work already overlapped with v) + delta seed
                for j in range(NCH):
                    nc.gpsimd.tensor_add(
                        out=delta[j], in0=x_t[j], in1=mbr_b
                    )
                    xab = ln_chunk(x_d, x_t[j], qg_b, qb_b, tA, "lx")
                    transpose_chunk(xab, xaT, j, ptp)
                proj_T(qw_sb, xaT, qT, qbias_c, pmm)

            # ---------------- phase B: per-head-pair attention + merge
            # software-pipelined: S+exp for pair hp while pair hp-1 does
            # den/outT/merge, so PE never waits on the Scalar Exp batch.
            with (
                tc.tile_pool(name="phB", bufs=2) as tB,
                tc.tile_pool(name="phBs", bufs=2) as tBs,
                tc.tile_pool(name="pmmB", bufs=6, space="PSUM") as pmm,
                tc.tile_pool(name="psS", bufs=2, space="PSUM") as psS,
            ):
                expS_p = {}

                def emit_S(hp):
                    cc_h = hp
                    expS = [
                        tB.tile([P, NCH, N], F8, tag=f"expS{par}",
                                name=f"expS{hp}_{par}")
                        for par in range(2)
                    ]
                    for i in range(NCH):
                        for ns in range(NS):
                            pse = [None, None]
                            for par in range(2):
                                po = par * DH
                                pse[par] = psS.tile([P, FD], F32, tag="psS",
                                                    name=f"psS{par}")
                                nc.tensor.matmul(
                                    pse[par],
                                    lhsT=kT[po:po + DH, cc_h, i * P:(i + 1) * P],
                                    rhs=qT[po:po + DH, cc_h,
                                           ns * FD:(ns + 1) * FD],
                                    start=True, stop=True,
                                )
                            for par in range(2):
                                nc.scalar.activation(
                                    out=expS[par][:, i, ns * FD:(ns + 1) * FD],
                                    in_=pse[par], func=AF.Exp, scale=SCALE,
                                )
                    expS_p[hp] = expS

                def emit_attn(hp):
                    for par in range(2):
                        h = 2 * hp + par
                        eS = expS_p[hp][par]
                        den_row = tBs.tile([1, N], F32, tag=f"denrow{par}",
                                           name=f"denrow{par}")
                        for ns in range(NS):
                            psd = pmm.tile([1, FD], F32, tag="pmm",
                                            name="psd")
                            for g in range(NCH // 2):
                                nc.tensor.matmul(
                                    psd,
                                    lhsT=ones8[:, :, 0:1],
                                    rhs=eS[:, 2 * g:2 * g + 2,
                                           ns * FD:(ns + 1) * FD],
                                    start=(g == 0), stop=(g == NCH // 2 - 1),
                                    perf_mode=DR,
                                )
                            nc.vector.tensor_scalar_mul(
                                out=den_row[0:1, ns * FD:(ns + 1) * FD],
                                in0=psd, scalar1=64.0,
                            )
                        recip_col = tBs.tile([P, NCH], F32, tag=f"recip{par}",
                                             name=f"recip{par}")
                        for j in range(NCH):
                            ptd = pmm.tile([P, 1], F32, tag="pmm", name="ptd")
                            nc.tensor.matmul(
                                ptd,
                                lhsT=den_row[0:1, j * P:(j + 1) * P],
                                rhs=ones_col[0:1, 0:1],
                                start=True, stop=True,
                            )
                            nc.vector.tensor_copy(
                                out=recip_col[:, j:j + 1], in_=ptd
                            )
                        nc.vector.reciprocal(out=recip_col, in_=recip_col)

                        # outT: ns interleaved inside cc so every LDWEIGHTS
                        # hides under the previous matmul of the same lhsT
                        outT_h = tB.tile([P, KC, N], F8, tag=f"outT{par}",
                                         name=f"outT{par}")
                        for cc in range(KC):
                            pso = [None, None]
                            for g in range(NCH // 2):
                                for ns in range(NS):
                                    if g == 0:
                                        pso[ns] = pmm.tile(
                                            [P, FD], F32, tag="pmm",
                                            name=f"pso{ns}")
                                    nc.tensor.matmul(
                                        pso[ns],
                                        lhsT=vv[h][:, 2 * g:2 * g + 2,
                                                   cc * P:(cc + 1) * P],
                                        rhs=eS[:, 2 * g:2 * g + 2,
                                               ns * FD:(ns + 1) * FD],
                                        start=(g == 0),
                                        stop=(g == NCH // 2 - 1),
                                        perf_mode=DR,
                                    )
                            for ns in range(NS):
                                nc.vector.tensor_scalar_mul(
                                    out=outT_h[:, cc, ns * FD:(ns + 1) * FD],
                                    in0=pso[ns], scalar1=1.0 / 32.0,
                                )

                        for j in range(NCH):
                            ps = pmm.tile([P, FD], F32, tag="pmm", name="ps")
                            for g in range(KC // 2):
                                nc.tensor.matmul(
                                    ps,
                                    lhsT=outT_h[:, 2 * g:2 * g + 2,
                                                j * P:(j + 1) * P],
                                    rhs=mw_sb[:, h * KC + 2 * g:
                                              h * KC + 2 * g + 2, :],
                                    start=(g == 0), stop=(g == KC // 2 - 1),
                                    perf_mode=DR,
                                )
                            dn = tBs.tile([P, FD], F32, tag="dnorm",
                                          name="dn")
                            nc.vector.tensor_scalar_mul(
                                out=dn, in0=ps, scalar1=recip_col[:, j:j + 1]
                            )
                            nc.gpsimd.tensor_add(
                                out=delta[j], in0=delta[j], in1=dn
                            )

                for hp in range(H // 2 + 1):
                    if hp <= H // 2 - 1:
                        emit_S(hp)
                    if hp > 0:
                        emit_attn(hp - 1)
                        del expS_p[hp - 1]

                # x2 LayerNorm emitted here so it overlaps the tail merges
                for j in range(NCH):
                    nc.gpsimd.tensor_add(out=x2b[j], in0=delta[j], in1=fb2_b)
                    ln_chunk(None, delta[j], ffg_b, ffb_b, tBs, "f",
                             out_t=fabs[j])

            acts_ab_cm.__exit__(None, None, None)

            # ---------------- phase C: FFN + output
            with (
                tc.tile_pool(name="phC", bufs=3) as tC,
                tc.tile_pool(name="phCl", bufs=1) as tCl,
                tc.tile_pool(name="pmmC", bufs=4, space="PSUM") as pmm,
                tc.tile_pool(name="ptC", bufs=2, space="PSUM") as ptp,
            ):
                ffaT = tCl.tile([P, KC, N], F8, tag="ffaT")
                for j in range(NCH):
                    transpose_chunk(fabs[j], ffaT, j, ptp)

                # h1T = swish(ff_w1.T @ ffaT / WS + b1)   [e, n] fp8
                haT = tCl.tile([P, ECH, N], F8, tag="haT")
                for ec in range(ECH):
                    psh = [None, None]
                    for g in range(KC // 2):
                        for ns in range(NS):
                            if g == 0:
                                psh[ns] = pmm.tile([P, FD], F32, tag="pmm",
                                                   name=f"psh{ns}")
                            nc.tensor.matmul(
                                psh[ns],
                                lhsT=fw1_sb[:, 2 * g:2 * g + 2,
                                            ec * P:(ec + 1) * P],
                                rhs=ffaT[:, 2 * g:2 * g + 2,
                                         ns * FD:(ns + 1) * FD],
                                start=(g == 0), stop=(g == KC // 2 - 1),
                                perf_mode=DR,
                            )
                    for ns in range(NS):
                        nc.scalar.activation(
                            out=haT[:, ec, ns * FD:(ns + 1) * FD],
                            in_=psh[ns], func=AF.Silu, scale=1.0 / WS,
                            bias=fb1_c[:, ec:ec + 1],
                        )

                # ff natural [n, c]: out = ps/WS + (x2 + b2)
                for j in range(NCH):
                    ps = pmm.tile([P, FD], F32, tag="pmm", name="ps")
                    for g in range(ECH // 2):
                        nc.tensor.matmul(
                            ps,
                            lhsT=haT[:, 2 * g:2 * g + 2, j * P:(j + 1) * P],
                            rhs=fw2_sb[:, 2 * g:2 * g + 2, :],
                            start=(g == 0), stop=(g == ECH // 2 - 1),
                            perf_mode=DR,
                        )
                    ot = tC.tile([P, D], F32, tag="ot", name="ot")
                    nc.scalar.activation(
                        out=ot, in_=ps, func=AF.Copy, scale=1.0 / WS
                    )
                    nc.vector.tensor_add(out=ot, in0=ot, in1=x2b[j])
                    nc.sync.dma_start(out_d.ap()[j * P:(j + 1) * P, :], ot)

            phBC_cm.__exit__(None, None, None)
            phCw_cm.__exit__(None, None, None)

    return nc


_CACHED = {}


def _get_nc(skip_gb):
    key = f"nc_{skip_gb}"
    if key not in _CACHED:
        _install_compat()
        _CACHED[key] = _build(skip_gb=skip_gb)
    return _CACHED[key]


def kernel(**inputs):
    skip_gb = all(
        np.all(np.asarray(inputs[g]) == 1.0) and np.all(np.asarray(inputs[b]) == 0.0)
        for g, b in (("q_g", "q_b"), ("kv_g", "kv_b"), ("ff_g", "ff_b"))
    )
    nc = _get_nc(skip_gb)
    b = inputs["x"].shape[0]
    assert b == 8
    import ml_dtypes
    fp8_names = {"q_w", "kv_w", "merge_w", "ff_w1", "ff_w2"}
    shared = {}
    for k, v in inputs.items():
        if k in ("x", "context", "merge_b"):
            continue
        v = np.asarray(v)
        if k in fp8_names:
            sv = np.clip(v.astype(np.float64) * WS, -240.0, 240.0)
            shared[k] = np.ascontiguousarray(sv.astype(ml_dtypes.float8_e4m3))
        else:
            shared[k] = np.ascontiguousarray(v.astype(np.float32))
    # fold the v-projection bias through merge_w into the residual seed:
    # attn @ (v + b) @ W == attn @ v @ W + b @ W  (attn rows sum to 1)
    vb = np.asarray(inputs["kv_bias"]).astype(np.float64)[DH * H:]
    mw = np.asarray(inputs["merge_w"]).astype(np.float64)
    mbr = np.asarray(inputs["merge_b"]).astype(np.float64) + vb @ mw
    shared["mbr"] = np.ascontiguousarray(mbr.astype(np.float32))
    in_maps = []
    for i in range(b):
        m = dict(shared)
        m["x"] = np.ascontiguousarray(np.asarray(inputs["x"][i], dtype=np.float32))
        m["context"] = np.ascontiguousarray(
            np.asarray(inputs["context"][i], dtype=np.float32)
        )
        in_maps.append(m)
    res = run_bass_kernel_spmd(nc, in_maps, core_ids=list(range(8)))
    _CACHED["last_results"] = res
    return np.stack([res.results[i]["out"] for i in range(8)])


# revision 12
# speedup vs baseline: 1.0550x; 1.0550x over previous
"""CrossAttnBlock kernel for 8 Trainium2 NeuronCores.

Sharding: data-parallel over the batch dim B=8 -> one batch item per core.
Each core runs the full block (q/kv projections, cross-attention, merge,
FFN) on its [1024, 512] slice; weights are replicated.

v2: fp8 (e4m3) DoubleRow matmuls for all dense GEMMs -- each matmul
instruction contracts 2x128 K-rows (weights host-scaled by 256 to sit in
fp8 normal range; the 1/256 is folded into the PSUM readout scale).
Attention S^T stays bf16 (K=64) but head PAIRS are issued back-to-back so
even/odd heads run concurrently on PE row-groups 0-63 / 64-127.
The per-head v bias is folded host-side into the residual seed
(attn rows sum to 1 after normalization, so attn@(v+b)@W = attn@v@W + b@W).

Layout (per core): activations transposed so every matmul contracts over
the partition dim:
  xaT/ctxaT [d, n] fp8   (LN+swish fused: silu((x-mu)*rs) via act scale/bias)
  qT/kT     [c, n] bf16  (feed S only)
  vv_h      [m, c] fp8   (scaled 8x: readout 256*v -> *1/32)
  S^T[m, n] -> expS fp8 = exp(S*scale)
  den       = onesT @ expS (fp8 DR), *64 at readout; recip includes /64
  outT[c,n] fp8 = (v.T @ expS)/32  (so merge PSUM = 64*delta_unnorm)
  delta[n,c] += (outT.T @ mw) * recip        (mul on DVE, add on GpSimd)
  FFN: LN/swish -> transpose -> h1T (silu(ps/256+b1)) -> ff + residual
"""

import json

import numpy as np

import concourse.bass as bass
import concourse.mybir as mybir
import concourse.tile as tile
from concourse.bass_utils import run_bass_kernel_spmd

F32 = mybir.dt.float32
BF16 = mybir.dt.bfloat16
F8 = mybir.dt.float8e4
AF = mybir.ActivationFunctionType
DR = mybir.MatmulPerfMode.DoubleRow

P = 128
N = 1024          # query rows per core
M = 1024          # context rows per core
D = 512           # d_in == d_ctx == d_out
H = 8             # heads
DH = 64           # head dim (k/q)
DE = 2048         # ffn expand
KC = D // P       # 4 feature chunks
NCH = N // P      # 8 row chunks
ECH = DE // P     # 16 expand chunks
SCALE = DH ** -0.5
EPS = 1e-5
NS = 2            # free-dim split of 1024 into 2x512
FD = 512          # matmul moving free dim
WS = 256.0        # host-side fp8 weight scale


# --- workaround: this walrus build allows only ONE embedded sync wait per
# instruction. Hoist all but the last wait of every instruction onto
# preceding single-wait NoOps on the same engine.

def _split_multiwait_drains(bir_json: bytes) -> bytes:
    d = json.loads(bir_json)
    changed = False
    for fn in d.get("functions", []):
        for blk in fn.get("blocks", []):
            out = []
            for inst in blk.get("instructions", []):
                si = inst.get("sync_info") or {}
                waits = si.get("on_wait") or []
                if len(waits) > 1:
                    for j, w in enumerate(waits[:-1]):
                        out.append({
                            "name": f"{inst['name']}__w{j}",
                            "engine": inst["engine"],
                            "opcode": "NoOp",
                            "ins": [],
                            "outs": [],
                            "debug": inst.get("debug"),
                            "sync_info": {"on_wait": [w], "on_update": []},
                        })
                    si["on_wait"] = [waits[-1]]
                    changed = True
                out.append(inst)
            blk["instructions"] = out
    if not changed:
        return bir_json
    return json.dumps(d).encode()


def _install_compat():
    import concourse.bass_utils as bu
    import concourse.bass2jax as b2j

    if getattr(b2j, "_drain_split_installed", False):
        return
    orig = bu.compile_bir_kernel

    def patched(bir_json, tmpdir, neff_name="file.neff"):
        return orig(_split_multiwait_drains(bir_json), tmpdir, neff_name)

    b2j.compile_bir_kernel = patched
    b2j._drain_split_installed = True


def _bcast_1d(t, n):
    """DRAM [n] vector -> AP broadcast to [P, n] (partition stride 0)."""
    ap = t.ap()
    return bass.AP(tensor=ap.tensor, offset=ap.offset, ap=[[0, P], ap.ap[0]])


def _build(skip_gb=False):
    nc = bass.Bass("TRN2")

    x_d = nc.dram_tensor("x", [N, D], F32, kind="ExternalInput")
    ctx_d = nc.dram_tensor("context", [M, D], F32, kind="ExternalInput")
    qg_d = nc.dram_tensor("q_g", [D], F32, kind="ExternalInput")
    qb_d = nc.dram_tensor("q_b", [D], F32, kind="ExternalInput")
    qw_d = nc.dram_tensor("q_w", [D, DH * H], F8, kind="ExternalInput")
    qbias_d = nc.dram_tensor("q_bias", [DH * H], F32, kind="ExternalInput")
    kvg_d = nc.dram_tensor("kv_g", [D], F32, kind="ExternalInput")
    kvb_d = nc.dram_tensor("kv_b", [D], F32, kind="ExternalInput")
    kvw_d = nc.dram_tensor("kv_w", [D, (DH + D) * H], F8, kind="ExternalInput")
    kvbias_d = nc.dram_tensor("kv_bias", [(DH + D) * H], F32, kind="ExternalInput")
    mw_d = nc.dram_tensor("merge_w", [D * H, D], F8, kind="ExternalInput")
    mbr_d = nc.dram_tensor("mbr", [D], F32, kind="ExternalInput")
    ffg_d = nc.dram_tensor("ff_g", [D], F32, kind="ExternalInput")
    ffb_d = nc.dram_tensor("ff_b", [D], F32, kind="ExternalInput")
    fw1_d = nc.dram_tensor("ff_w1", [D, DE], F8, kind="ExternalInput")
    fb1_d = nc.dram_tensor("ff_b1", [DE], F32, kind="ExternalInput")
    fw2_d = nc.dram_tensor("ff_w2", [DE, D], F8, kind="ExternalInput")
    fb2_d = nc.dram_tensor("ff_b2", [D], F32, kind="ExternalInput")
    out_d = nc.dram_tensor("out", [N, D], F32, kind="ExternalOutput")

    from concourse.masks import make_identity

    with tile.TileContext(nc) as tc:
        with (
            tc.tile_pool(name="persist", bufs=1) as pers,
            tc.tile_pool(name="resid", bufs=1) as resid_pool,
        ):
            ident = pers.tile([P, P], F32, tag="ident")
            make_identity(nc, ident)
            ident_bf = pers.tile([P, P], BF16, tag="ident_bf")
            nc.vector.tensor_copy(out=ident_bf, in_=ident)
            eps_t = pers.tile([P, 1], F32, tag="eps")
            nc.vector.memset(eps_t, EPS)
            ones_col = pers.tile([P, 1], F32, tag="ones")
            nc.vector.memset(ones_col, 1.0)
            # fp8 ones for the DoubleRow denominator matmul: lhsT must be a
            # 3D AP [128, 2, 1] whose middle-dim byte step is 16-aligned.
            ones8 = pers.tile([P, 2, 16], F8, tag="ones8")
            nc.vector.memset(ones8, 1.0)

            delta = [
                resid_pool.tile([P, D], F32, tag=f"delta{j}", name=f"delta{j}")
                for j in range(NCH)
            ]

            # per-partition-column biases
            with nc.allow_non_contiguous_dma(reason="tiny bias gathers"):
                qbias_c = pers.tile([P, KC], F32, tag="qbias")
                nc.gpsimd.dma_start(qbias_c, qbias_d.ap().rearrange("(o p) -> p o", p=P))
                kvbk_c = pers.tile([P, KC], F32, tag="kvbk")
                nc.gpsimd.dma_start(
                    kvbk_c, kvbias_d.ap()[0:DH * H].rearrange("(o p) -> p o", p=P)
                )
                fb1_c = pers.tile([P, ECH], F32, tag="fb1")
                nc.gpsimd.dma_start(fb1_c, fb1_d.ap().rearrange("(o p) -> p o", p=P))

            def ln_stats(xt, mvs, rss, j, pool, tag):
                """bn stats for one chunk; mean/var into mvs[:, 2j:2j+2]."""
                st = pool.tile([P, 6], F32, tag=f"{tag}_st", name="st")
                nc.vector.bn_stats(out=st, in_=xt)
                nc.vector.bn_aggr(out=mvs[:, 2 * j:2 * j + 2], in_=st)

            def ln_finish(mvs, rss, pool, tag):
                """batched rs = 1/sqrt(var+eps) for all NCH chunks."""
                nc.scalar.activation(
                    out=rss, in_=mvs[:, 1:2 * NCH:2], func=AF.Sqrt, bias=eps_t,
                )
                nc.vector.reciprocal(out=rss, in_=rss)

            def ln_apply(xt, mvs, rss, j, g_b, b_b, pool, tag, out_t=None):
                xab = out_t if out_t is not None else pool.tile(
                    [P, D], BF16, tag=f"{tag}_xab", name="xab")
                xa = pool.tile([P, D], F32, tag=f"{tag}_xa", name="xa")
                nc.vector.tensor_scalar(
                    out=xa, in0=xt, scalar1=mvs[:, 2 * j:2 * j + 1],
                    scalar2=rss[:, j:j + 1],
                    op0=mybir.AluOpType.subtract,
                    op1=mybir.AluOpType.mult,
                )
                if not skip_gb:
                    nc.gpsimd.tensor_mul(out=xa, in0=xa, in1=g_b)
                    nc.gpsimd.tensor_add(out=xa, in0=xa, in1=b_b)
                nc.scalar.activation(out=xab, in_=xa, func=AF.Silu)
                return xab

            def transpose_chunk(xab, dstT, j, psum_t):
                for kc in range(KC):
                    pt = psum_t.tile([P, P], BF16, tag="pt", name="pt")
                    nc.tensor.transpose(pt, xab[:, kc * P:(kc + 1) * P], ident_bf)
                    nc.vector.tensor_copy(
                        out=dstT[:, kc, j * P:(j + 1) * P], in_=pt
                    )

            def proj_T(w_sb, rhsT, dst, bias_c, pmm):
                """dst[c, n] = (w.T @ act)/WS + bias, bf16 (DoubleRow)."""
                for cc in range(KC):
                    for ns in range(NS):
                        ps = pmm.tile([P, FD], F32, tag="pmm", name="ps")
                        for g in range(KC // 2):
                            nc.tensor.matmul(
                                ps,
                                lhsT=w_sb[:, 2 * g:2 * g + 2, cc * P:(cc + 1) * P],
                                rhs=rhsT[:, 2 * g:2 * g + 2, ns * FD:(ns + 1) * FD],
                                start=(g == 0), stop=(g == KC // 2 - 1),
                                perf_mode=DR,
                            )
                        nc.vector.tensor_scalar(
                            out=dst[:, cc, ns * FD:(ns + 1) * FD],
                            in0=ps, scalar1=1.0 / WS, scalar2=bias_c[:, cc:cc + 1],
                            op0=mybir.AluOpType.mult, op1=mybir.AluOpType.add,
                        )

            # ---- weight tiles for phase C (allocated early, DMA'd last)
            phCw_cm = tc.tile_pool(name="phCw", bufs=1)
            tCw = phCw_cm.__enter__()
            fw1_sb = tCw.tile([P, KC, DE], F8, tag="fw1")
            fw2_sb = tCw.tile([P, ECH, D], F8, tag="fw2")
            fb2_b = tCw.tile([P, D], F32, tag="fb2")
            if not skip_gb:
                ffg_b = tCw.tile([P, D], F32, tag="ffg")
                ffb_b = tCw.tile([P, D], F32, tag="ffb")
            else:
                ffg_b = ffb_b = None

            # LN(x2) results, produced in phase B tail, used in phase C
            phBC_cm = tc.tile_pool(name="phBC", bufs=1)
            tBC = phBC_cm.__enter__()
            fabs = [tBC.tile([P, D], BF16, tag=f"fab{j}", name=f"fab{j}")
                    for j in range(NCH)]
            x2b = [tBC.tile([P, D], F32, tag=f"x2b{j}", name=f"x2b{j}")
                   for j in range(NCH)]

            # ---- activations + weights that span phases A+B
            acts_ab_cm = tc.tile_pool(name="actsAB", bufs=1)
            acts_ab = acts_ab_cm.__enter__()
            ctxaT = acts_ab.tile([P, KC, M], F8, tag="ctxaT")
            qT = acts_ab.tile([P, KC, N], BF16, tag="qT")
            kT = acts_ab.tile([P, KC, M], BF16, tag="kT")
            vv = [acts_ab.tile([P, NCH, D], F8, tag=f"vv{h}", name=f"vv{h}")
                  for h in range(H)]
            mw_sb = acts_ab.tile([P, KC * H, D], F8, tag="mw")

            # ---------------- phase A: LN/swish/transpose + projections + v
            with (
                tc.tile_pool(name="phA", bufs=3) as tA,
                tc.tile_pool(name="phA_in", bufs=1) as tAin,
                tc.tile_pool(name="phA_w", bufs=1) as tAw,
                tc.tile_pool(name="pmmA", bufs=4, space="PSUM") as pmm,
                tc.tile_pool(name="ptA", bufs=2, space="PSUM") as ptp,
            ):
                xaT = tAw.tile([P, KC, N], F8, tag="xaT")
                kvwv_sb = tAw.tile([P, KC, D * H], F8, tag="kvwv")
                qw_sb = tAw.tile([P, KC, DH * H], F8, tag="qw")
                kvwk_sb = tAw.tile([P, KC, DH * H], F8, tag="kvwk")
                mbr_b = tAw.tile([P, D], F32, tag="mbr")
                if not skip_gb:
                    qg_b = tAw.tile([P, D], F32, tag="qg")
                    nc.gpsimd.dma_start(qg_b, _bcast_1d(qg_d, D))
                    qb_b = tAw.tile([P, D], F32, tag="qb")
                    nc.gpsimd.dma_start(qb_b, _bcast_1d(qb_d, D))
                    kvg_b = tAw.tile([P, D], F32, tag="kvg")
                    nc.gpsimd.dma_start(kvg_b, _bcast_1d(kvg_d, D))
                    kvb_b = tAw.tile([P, D], F32, tag="kvb")
                    nc.gpsimd.dma_start(kvb_b, _bcast_1d(kvb_d, D))
                    nc.gpsimd.dma_start(ffg_b, _bcast_1d(ffg_d, D))
                    nc.gpsimd.dma_start(ffb_b, _bcast_1d(ffb_d, D))
                else:
                    qg_b = qb_b = kvg_b = kvb_b = None
                nc.gpsimd.dma_start(mbr_b, _bcast_1d(mbr_d, D))
                nc.gpsimd.dma_start(fb2_b, _bcast_1d(fb2_d, D))

                # one DMA queue, priority order: ctx, k/q weights, v weights,
                # x, merge weights, ffn weights -- so nothing starves ctx.
                ctx_t = []
                for j in range(NCH):
                    xt = tAin.tile([P, D], F32, tag=f"lc_in{j}", name="xt")
                    nc.sync.dma_start(xt, ctx_d.ap()[j * P:(j + 1) * P, :])
                    ctx_t.append(xt)
                nc.sync.dma_start(
                    kvwk_sb,
                    kvw_d.ap()[:, 0:DH * H].rearrange("(o p) c -> p o c", p=P),
                )
                nc.sync.dma_start(qw_sb, qw_d.ap().rearrange("(o p) c -> p o c", p=P))
                nc.sync.dma_start(
                    kvwv_sb,
                    kvw_d.ap()[:, DH * H:].rearrange("(o p) c -> p o c", p=P),
                )
                x_t = []
                for j in range(NCH):
                    xt = tAin.tile([P, D], F32, tag=f"lx_in{j}", name="xt2")
                    nc.sync.dma_start(xt, x_d.ap()[j * P:(j + 1) * P, :])
                    x_t.append(xt)
                nc.sync.dma_start(
                    mw_sb, mw_d.ap().rearrange("(o p) c -> p o c", p=P)
                )
                nc.sync.dma_start(
                    fw1_sb, fw1_d.ap().rearrange("(o p) c -> p o c", p=P)
                )
                nc.sync.dma_start(
                    fw2_sb, fw2_d.ap().rearrange("(o p) c -> p o c", p=P)
                )

                mvs_c = tAw.tile([P, 2 * NCH], F32, tag="mvs_c")
                rss_c = tAw.tile([P, NCH], F32, tag="rss_c")
                for j in range(NCH):
                    ln_stats(ctx_t[j], mvs_c, rss_c, j, tA, "lc")
                ln_finish(mvs_c, rss_c, tA, "lc")
                for j in range(NCH):
                    xab = ln_apply(ctx_t[j], mvs_c, rss_c, j, kvg_b, kvb_b,
                                   tA, "lc")
                    transpose_chunk(xab, ctxaT, j, ptp)
                proj_T(kvwk_sb, ctxaT, kT, kvbk_c, pmm)

                # v for all heads (scaled 8x), readouts split Scalar/DVE
                for h in range(H):
                    for i in range(NCH):
                        ps = pmm.tile([P, FD], F32, tag="pmm", name="ps")
                        for g in range(KC // 2):
                            nc.tensor.matmul(
                                ps,
                                lhsT=ctxaT[:, 2 * g:2 * g + 2, i * P:(i + 1) * P],
                                rhs=kvwv_sb[:, 2 * g:2 * g + 2,
                                            h * D:(h + 1) * D],
                                start=(g == 0), stop=(g == KC // 2 - 1),
                                perf_mode=DR,
                            )
                        if h % 2 == 0:
                            nc.scalar.activation(
                                out=vv[h][:, i, :], in_=ps, func=AF.Copy,
                                scale=8.0 / WS,
                            )
                        else:
                            nc.vector.tensor_scalar_mul(
                                out=vv[h][:, i, :], in0=ps, scalar1=8.0 / WS,
                            )

                # x LN (DVE work already overlapped with v) + delta seed
                mvs_x = tAw.tile([P, 2 * NCH], F32, tag="mvs_x")
                rss_x = tAw.tile([P, NCH], F32, tag="rss_x")
                for j in range(NCH):
                    nc.gpsimd.tensor_add(
                        out=delta[j], in0=x_t[j], in1=mbr_b
                    )
                    ln_stats(x_t[j], mvs_x, rss_x, j, tA, "lx")
                ln_finish(mvs_x, rss_x, tA, "lx")
                for j in range(NCH):
                    xab = ln_apply(x_t[j], mvs_x, rss_x, j, qg_b, qb_b,
                                   tA, "lx")
                    transpose_chunk(xab, xaT, j, ptp)
                proj_T(qw_sb, xaT, qT, qbias_c, pmm)

            # ---------------- phase B: per-head-pair attention + merge
            # software-pipelined: S+exp for pair hp while pair hp-1 does
            # den/outT/merge, so PE never waits on the Scalar Exp batch.
            with (
                tc.tile_pool(name="phB", bufs=2) as tB,
                tc.tile_pool(name="phBs", bufs=2) as tBs,
                tc.tile_pool(name="pmmB", bufs=4, space="PSUM") as pmm,
                tc.tile_pool(name="psS", bufs=2, space="PSUM") as psS,
                tc.tile_pool(name="pdenB", bufs=1, space="PSUM") as pden,
            ):
                expS_p = {}

                def emit_S(hp):
                    cc_h = hp
                    expS = [
                        tB.tile([P, NCH, N], F8, tag=f"expS{par}",
                                name=f"expS{hp}_{par}")
                        for par in range(2)
                    ]
                    for i in range(NCH):
                        for ns in range(NS):
                            pse = [None, None]
                            for par in range(2):
                                po = par * DH
                                pse[par] = psS.tile([P, FD], F32, tag="psS",
                                                    name=f"psS{par}")
                                nc.tensor.matmul(
                                    pse[par],
                                    lhsT=kT[po:po + DH, cc_h, i * P:(i + 1) * P],
                                    rhs=qT[po:po + DH, cc_h,
                                           ns * FD:(ns + 1) * FD],
                                    start=True, stop=True,
                                )
                            for par in range(2):
                                nc.scalar.activation(
                                    out=expS[par][:, i, ns * FD:(ns + 1) * FD],
                                    in_=pse[par], func=AF.Exp, scale=SCALE,
                                )
                    expS_p[hp] = expS

                def emit_attn(hp):
                    for par in range(2):
                        h = 2 * hp + par
                        eS = expS_p[hp][par]
                        den_row = tBs.tile([1, N], F32, tag=f"denrow{par}",
                                           name=f"denrow{par}")
                        for ns in range(NS):
                            psd = pden.tile([1, FD], F32, tag="pden",
                                            name="psd")
                            for g in range(NCH // 2):
                                nc.tensor.matmul(
                                    psd,
                                    lhsT=ones8[:, :, 0:1],
                                    rhs=eS[:, 2 * g:2 * g + 2,
                                           ns * FD:(ns + 1) * FD],
                                    start=(g == 0), stop=(g == NCH // 2 - 1),
                                    perf_mode=DR,
                                )
                            nc.vector.tensor_scalar_mul(
                                out=den_row[0:1, ns * FD:(ns + 1) * FD],
                                in0=psd, scalar1=64.0,
                            )
                        recip_col = tBs.tile([P, NCH], F32, tag=f"recip{par}",
                                             name=f"recip{par}")
                        for j in range(NCH):
                            ptd = pden.tile([P, 1], F32, tag="ptd", name="ptd")
                            nc.tensor.matmul(
                                ptd,
                                lhsT=den_row[0:1, j * P:(j + 1) * P],
                                rhs=ones_col[0:1, 0:1],
                                start=True, stop=True,
                            )
                            nc.vector.tensor_copy(
                                out=recip_col[:, j:j + 1], in_=ptd
                            )
                        nc.vector.reciprocal(out=recip_col, in_=recip_col)

                        # outT: ns interleaved inside cc so every LDWEIGHTS
                        # hides under the previous matmul of the same lhsT
                        outT_h = tB.tile([P, KC, N], F8, tag=f"outT{par}",
                                         name=f"outT{par}")
                        for cc in range(KC):
                            pso = [None, None]
                            for g in range(NCH // 2):
                                for ns in range(NS):
                                    if g == 0:
                                        pso[ns] = pmm.tile(
                                            [P, FD], F32, tag="pmm",
                                            name=f"pso{ns}")
                                    nc.tensor.matmul(
                                        pso[ns],
                                        lhsT=vv[h][:, 2 * g:2 * g + 2,
                                                   cc * P:(cc + 1) * P],
                                        rhs=eS[:, 2 * g:2 * g + 2,
                                               ns * FD:(ns + 1) * FD],
                                        start=(g == 0),
                                        stop=(g == NCH // 2 - 1),
                                        perf_mode=DR,
                                    )
                            for ns in range(NS):
                                nc.vector.tensor_scalar_mul(
                                    out=outT_h[:, cc, ns * FD:(ns + 1) * FD],
                                    in0=pso[ns], scalar1=1.0 / 32.0,
                                )

                        for j in range(NCH):
                            ps = pmm.tile([P, FD], F32, tag="pmm", name="ps")
                            for g in range(KC // 2):
                                nc.tensor.matmul(
                                    ps,
                                    lhsT=outT_h[:, 2 * g:2 * g + 2,
                                                j * P:(j + 1) * P],
                                    rhs=mw_sb[:, h * KC + 2 * g:
                                              h * KC + 2 * g + 2, :],
                                    start=(g == 0), stop=(g == KC // 2 - 1),
                                    perf_mode=DR,
                                )
                            dn = tBs.tile([P, FD], F32, tag="dnorm",
                                          name="dn")
                            nc.vector.tensor_scalar_mul(
                                out=dn, in0=ps, scalar1=recip_col[:, j:j + 1]
                            )
                            nc.gpsimd.tensor_add(
                                out=delta[j], in0=delta[j], in1=dn
                            )

                for hp in range(H // 2 + 1):
                    if hp <= H // 2 - 1:
                        emit_S(hp)
                    if hp > 0:
                        emit_attn(hp - 1)
                        del expS_p[hp - 1]

                # x2 LayerNorm emitted here so it overlaps the tail merges
                mvs_f = tBs.tile([P, 2 * NCH], F32, tag="mvs_f")
                rss_f = tBs.tile([P, NCH], F32, tag="rss_f")
                for j in range(NCH):
                    nc.gpsimd.tensor_add(out=x2b[j], in0=delta[j], in1=fb2_b)
                    ln_stats(delta[j], mvs_f, rss_f, j, tBs, "f")
                ln_finish(mvs_f, rss_f, tBs, "f")
                for j in range(NCH):
                    ln_apply(delta[j], mvs_f, rss_f, j, ffg_b, ffb_b,
                             tBs, "f", out_t=fabs[j])

            acts_ab_cm.__exit__(None, None, None)

            # ---------------- phase C: FFN + output
            with (
                tc.tile_pool(name="phC", bufs=3) as tC,
                tc.tile_pool(name="phCl", bufs=1) as tCl,
                tc.tile_pool(name="pmmC", bufs=4, space="PSUM") as pmm,
                tc.tile_pool(name="ptC", bufs=2, space="PSUM") as ptp,
            ):
                ffaT = tCl.tile([P, KC, N], F8, tag="ffaT")
                for j in range(NCH):
                    transpose_chunk(fabs[j], ffaT, j, ptp)

                # h1T = swish(ff_w1.T @ ffaT / WS + b1)   [e, n] fp8
                haT = tCl.tile([P, ECH, N], F8, tag="haT")
                for ec in range(ECH):
                    psh = [None, None]
                    for g in range(KC // 2):
                        for ns in range(NS):
                            if g == 0:
                                psh[ns] = pmm.tile([P, FD], F32, tag="pmm",
                                                   name=f"psh{ns}")
                            nc.tensor.matmul(
                                psh[ns],
                                lhsT=fw1_sb[:, 2 * g:2 * g + 2,
                                            ec * P:(ec + 1) * P],
                                rhs=ffaT[:, 2 * g:2 * g + 2,
                                         ns * FD:(ns + 1) * FD],
                                start=(g == 0), stop=(g == KC // 2 - 1),
                                perf_mode=DR,
                            )
                    for ns in range(NS):
                        nc.scalar.activation(
                            out=haT[:, ec, ns * FD:(ns + 1) * FD],
                            in_=psh[ns], func=AF.Silu, scale=1.0 / WS,
                            bias=fb1_c[:, ec:ec + 1],
                        )

                # ff natural [n, c]: out = ps/WS + (x2 + b2)
                for j in range(NCH):
                    ps = pmm.tile([P, FD], F32, tag="pmm", name="ps")
                    for g in range(ECH // 2):
                        nc.tensor.matmul(
                            ps,
                            lhsT=haT[:, 2 * g:2 * g + 2, j * P:(j + 1) * P],
                            rhs=fw2_sb[:, 2 * g:2 * g + 2, :],
                            start=(g == 0), stop=(g == ECH // 2 - 1),
                            perf_mode=DR,
                        )
                    ot = tC.tile([P, D], F32, tag="ot", name="ot")
                    nc.scalar.activation(
                        out=ot, in_=ps, func=AF.Copy, scale=1.0 / WS
                    )
                    nc.vector.tensor_add(out=ot, in0=ot, in1=x2b[j])
                    nc.sync.dma_start(out_d.ap()[j * P:(j + 1) * P, :], ot)

            phBC_cm.__exit__(None, None, None)
            phCw_cm.__exit__(None, None, None)

    return nc


_CACHED = {}


def _get_nc(skip_gb):
    key = f"nc_{skip_gb}"
    if key not in _CACHED:
        _install_compat()
        _CACHED[key] = _build(skip_gb=skip_gb)
    return _CACHED[key]


def kernel(**inputs):
    skip_gb = all(
        np.all(np.asarray(inputs[g]) == 1.0) and np.all(np.asarray(inputs[b]) == 0.0)
        for g, b in (("q_g", "q_b"), ("kv_g", "kv_b"), ("ff_g", "ff_b"))
    )
    nc = _get_nc(skip_gb)
    b = inputs["x"].shape[0]
    assert b == 8
    import ml_dtypes
    fp8_names = {"q_w", "kv_w", "merge_w", "ff_w1", "ff_w2"}
    shared = {}
    for k, v in inputs.items():
        if k in ("x", "context", "merge_b"):
            continue
        v = np.asarray(v)
        if k in fp8_names:
            sv = np.clip(v.astype(np.float64) * WS, -240.0, 240.0)
            shared[k] = np.ascontiguousarray(sv.astype(ml_dtypes.float8_e4m3))
        else:
            shared[k] = np.ascontiguousarray(v.astype(np.float32))
    # fold the v-projection bias through merge_w into the residual seed:
    # attn @ (v + b) @ W == attn @ v @ W + b @ W  (attn rows sum to 1)
    vb = np.asarray(inputs["kv_bias"]).astype(np.float64)[DH * H:]
    mw = np.asarray(inputs["merge_w"]).astype(np.float64)
    mbr = np.asarray(inputs["merge_b"]).astype(np.float64) + vb @ mw
    shared["mbr"] = np.ascontiguousarray(mbr.astype(np.float32))
    in_maps = []
    for i in range(b):
        m = dict(shared)
        m["x"] = np.ascontiguousarray(np.asarray(inputs["x"][i], dtype=np.float32))
        m["context"] = np.ascontiguousarray(
            np.asarray(inputs["context"][i], dtype=np.float32)
        )
        in_maps.append(m)
    res = run_bass_kernel_spmd(nc, in_maps, core_ids=list(range(8)))
    _CACHED["last_results"] = res
    return np.stack([res.results[i]["out"] for i in range(8)])


# revision 13
# speedup vs baseline: 1.0676x; 1.0120x over previous
"""CrossAttnBlock kernel for 8 Trainium2 NeuronCores.

Sharding: data-parallel over the batch dim B=8 -> one batch item per core.
Each core runs the full block (q/kv projections, cross-attention, merge,
FFN) on its [1024, 512] slice; weights are replicated.

v2: fp8 (e4m3) DoubleRow matmuls for all dense GEMMs -- each matmul
instruction contracts 2x128 K-rows (weights host-scaled by 256 to sit in
fp8 normal range; the 1/256 is folded into the PSUM readout scale).
Attention S^T stays bf16 (K=64) but head PAIRS are issued back-to-back so
even/odd heads run concurrently on PE row-groups 0-63 / 64-127.
The per-head v bias is folded host-side into the residual seed
(attn rows sum to 1 after normalization, so attn@(v+b)@W = attn@v@W + b@W).

Layout (per core): activations transposed so every matmul contracts over
the partition dim:
  xaT/ctxaT [d, n] fp8   (LN+swish fused: silu((x-mu)*rs) via act scale/bias)
  qT/kT     [c, n] bf16  (feed S only)
  vv_h      [m, c] fp8   (scaled 8x: readout 256*v -> *1/32)
  S^T[m, n] -> expS fp8 = exp(S*scale)
  den       = onesT @ expS (fp8 DR), *64 at readout; recip includes /64
  outT[c,n] fp8 = (v.T @ expS)/32  (so merge PSUM = 64*delta_unnorm)
  delta[n,c] += (outT.T @ mw) * recip        (mul on DVE, add on GpSimd)
  FFN: LN/swish -> transpose -> h1T (silu(ps/256+b1)) -> ff + residual
"""

import json

import numpy as np

import concourse.bass as bass
import concourse.mybir as mybir
import concourse.tile as tile
from concourse.bass_utils import run_bass_kernel_spmd

F32 = mybir.dt.float32
BF16 = mybir.dt.bfloat16
F8 = mybir.dt.float8e4
AF = mybir.ActivationFunctionType
DR = mybir.MatmulPerfMode.DoubleRow

P = 128
N = 1024          # query rows per core
M = 1024          # context rows per core
D = 512           # d_in == d_ctx == d_out
H = 8             # heads
DH = 64           # head dim (k/q)
DE = 2048         # ffn expand
KC = D // P       # 4 feature chunks
NCH = N // P      # 8 row chunks
ECH = DE // P     # 16 expand chunks
SCALE = DH ** -0.5
EPS = 1e-5
NS = 2            # free-dim split of 1024 into 2x512
FD = 512          # matmul moving free dim
WS = 256.0        # host-side fp8 weight scale


# --- workaround: this walrus build allows only ONE embedded sync wait per
# instruction. Hoist all but the last wait of every instruction onto
# preceding single-wait NoOps on the same engine.

def _split_multiwait_drains(bir_json: bytes) -> bytes:
    d = json.loads(bir_json)
    changed = False
    for fn in d.get("functions", []):
        for blk in fn.get("blocks", []):
            out = []
            for inst in blk.get("instructions", []):
                si = inst.get("sync_info") or {}
                waits = si.get("on_wait") or []
                if len(waits) > 1:
                    for j, w in enumerate(waits[:-1]):
                        out.append({
                            "name": f"{inst['name']}__w{j}",
                            "engine": inst["engine"],
                            "opcode": "NoOp",
                            "ins": [],
                            "outs": [],
                            "debug": inst.get("debug"),
                            "sync_info": {"on_wait": [w], "on_update": []},
                        })
                    si["on_wait"] = [waits[-1]]
                    changed = True
                out.append(inst)
            blk["instructions"] = out
    if not changed:
        return bir_json
    return json.dumps(d).encode()


def _install_compat():
    import concourse.bass_utils as bu
    import concourse.bass2jax as b2j

    if getattr(b2j, "_drain_split_installed", False):
        return
    orig = bu.compile_bir_kernel

    def patched(bir_json, tmpdir, neff_name="file.neff"):
        return orig(_split_multiwait_drains(bir_json), tmpdir, neff_name)

    b2j.compile_bir_kernel = patched
    b2j._drain_split_installed = True


def _bcast_1d(t, n):
    """DRAM [n] vector -> AP broadcast to [P, n] (partition stride 0)."""
    ap = t.ap()
    return bass.AP(tensor=ap.tensor, offset=ap.offset, ap=[[0, P], ap.ap[0]])


def _build(skip_gb=False):
    nc = bass.Bass("TRN2")

    x_d = nc.dram_tensor("x", [N, D], F32, kind="ExternalInput")
    ctx_d = nc.dram_tensor("context", [M, D], F32, kind="ExternalInput")
    qg_d = nc.dram_tensor("q_g", [D], F32, kind="ExternalInput")
    qb_d = nc.dram_tensor("q_b", [D], F32, kind="ExternalInput")
    qw_d = nc.dram_tensor("q_w", [D, DH * H], F8, kind="ExternalInput")
    qbias_d = nc.dram_tensor("q_bias", [DH * H], F32, kind="ExternalInput")
    kvg_d = nc.dram_tensor("kv_g", [D], F32, kind="ExternalInput")
    kvb_d = nc.dram_tensor("kv_b", [D], F32, kind="ExternalInput")
    kvw_d = nc.dram_tensor("kv_w", [D, (DH + D) * H], F8, kind="ExternalInput")
    kvbias_d = nc.dram_tensor("kv_bias", [(DH + D) * H], F32, kind="ExternalInput")
    mw_d = nc.dram_tensor("merge_w", [D * H, D], F8, kind="ExternalInput")
    mbr_d = nc.dram_tensor("mbr", [D], F32, kind="ExternalInput")
    ffg_d = nc.dram_tensor("ff_g", [D], F32, kind="ExternalInput")
    ffb_d = nc.dram_tensor("ff_b", [D], F32, kind="ExternalInput")
    fw1_d = nc.dram_tensor("ff_w1", [D, DE], F8, kind="ExternalInput")
    fb1_d = nc.dram_tensor("ff_b1", [DE], F32, kind="ExternalInput")
    fw2_d = nc.dram_tensor("ff_w2", [DE, D], F8, kind="ExternalInput")
    fb2_d = nc.dram_tensor("ff_b2", [D], F32, kind="ExternalInput")
    out_d = nc.dram_tensor("out", [N, D], F32, kind="ExternalOutput")

    from concourse.masks import make_identity

    with tile.TileContext(nc) as tc:
        with (
            tc.tile_pool(name="persist", bufs=1) as pers,
            tc.tile_pool(name="resid", bufs=1) as resid_pool,
        ):
            ident = pers.tile([P, P], F32, tag="ident")
            make_identity(nc, ident)
            ident_bf = pers.tile([P, P], BF16, tag="ident_bf")
            nc.vector.tensor_copy(out=ident_bf, in_=ident)
            eps_t = pers.tile([P, 1], F32, tag="eps")
            nc.vector.memset(eps_t, EPS)
            ones_col = pers.tile([P, 1], F32, tag="ones")
            nc.vector.memset(ones_col, 1.0)
            # fp8 ones for the DoubleRow denominator matmul: lhsT must be a
            # 3D AP [128, 2, 1] whose middle-dim byte step is 16-aligned.
            ones8 = pers.tile([P, 2, 16], F8, tag="ones8")
            nc.vector.memset(ones8, 1.0)

            delta = [
                resid_pool.tile([P, D], F32, tag=f"delta{j}", name=f"delta{j}")
                for j in range(NCH)
            ]

            # per-partition-column biases
            with nc.allow_non_contiguous_dma(reason="tiny bias gathers"):
                qbias_c = pers.tile([P, KC], F32, tag="qbias")
                nc.gpsimd.dma_start(qbias_c, qbias_d.ap().rearrange("(o p) -> p o", p=P))
                kvbk_c = pers.tile([P, KC], F32, tag="kvbk")
                nc.gpsimd.dma_start(
                    kvbk_c, kvbias_d.ap()[0:DH * H].rearrange("(o p) -> p o", p=P)
                )
                fb1_c = pers.tile([P, ECH], F32, tag="fb1")
                nc.gpsimd.dma_start(fb1_c, fb1_d.ap().rearrange("(o p) -> p o", p=P))

            def ln_stats(xt, mvs, rss, j, pool, tag):
                """bn stats for one chunk; mean/var into mvs[:, 2j:2j+2]."""
                st = pool.tile([P, 6], F32, tag=f"{tag}_st", name="st")
                nc.vector.bn_stats(out=st, in_=xt)
                nc.vector.bn_aggr(out=mvs[:, 2 * j:2 * j + 2], in_=st)

            def ln_finish(mvs, rss, pool, tag, lo=0, hi=NCH):
                """batched rs = 1/sqrt(var+eps) for chunks [lo, hi)."""
                nc.scalar.activation(
                    out=rss[:, lo:hi], in_=mvs[:, 2 * lo + 1:2 * hi:2],
                    func=AF.Sqrt, bias=eps_t,
                )
                nc.vector.reciprocal(out=rss[:, lo:hi], in_=rss[:, lo:hi])

            def ln_apply(xt, mvs, rss, j, g_b, b_b, pool, tag, out_t=None):
                xab = out_t if out_t is not None else pool.tile(
                    [P, D], BF16, tag=f"{tag}_xab", name="xab")
                xa = pool.tile([P, D], F32, tag=f"{tag}_xa", name="xa")
                nc.vector.tensor_scalar(
                    out=xa, in0=xt, scalar1=mvs[:, 2 * j:2 * j + 1],
                    scalar2=rss[:, j:j + 1],
                    op0=mybir.AluOpType.subtract,
                    op1=mybir.AluOpType.mult,
                )
                if not skip_gb:
                    nc.gpsimd.tensor_mul(out=xa, in0=xa, in1=g_b)
                    nc.gpsimd.tensor_add(out=xa, in0=xa, in1=b_b)
                nc.scalar.activation(out=xab, in_=xa, func=AF.Silu)
                return xab

            def transpose_chunk(xab, dstT, j, psum_t):
                for kc in range(KC):
                    pt = psum_t.tile([P, P], BF16, tag="pt", name="pt")
                    nc.tensor.transpose(pt, xab[:, kc * P:(kc + 1) * P], ident_bf)
                    nc.vector.tensor_copy(
                        out=dstT[:, kc, j * P:(j + 1) * P], in_=pt
                    )

            def proj_T(w_sb, rhsT, dst, bias_c, pmm):
                """dst[c, n] = (w.T @ act)/WS + bias, bf16 (DoubleRow)."""
                for cc in range(KC):
                    for ns in range(NS):
                        ps = pmm.tile([P, FD], F32, tag="pmm", name="ps")
                        for g in range(KC // 2):
                            nc.tensor.matmul(
                                ps,
                                lhsT=w_sb[:, 2 * g:2 * g + 2, cc * P:(cc + 1) * P],
                                rhs=rhsT[:, 2 * g:2 * g + 2, ns * FD:(ns + 1) * FD],
                                start=(g == 0), stop=(g == KC // 2 - 1),
                                perf_mode=DR,
                            )
                        nc.vector.tensor_scalar(
                            out=dst[:, cc, ns * FD:(ns + 1) * FD],
                            in0=ps, scalar1=1.0 / WS, scalar2=bias_c[:, cc:cc + 1],
                            op0=mybir.AluOpType.mult, op1=mybir.AluOpType.add,
                        )

            # ---- weight tiles for phase C (allocated early, DMA'd last)
            phCw_cm = tc.tile_pool(name="phCw", bufs=1)
            tCw = phCw_cm.__enter__()
            fw1_sb = tCw.tile([P, KC, DE], F8, tag="fw1")
            fw2_sb = tCw.tile([P, ECH, D], F8, tag="fw2")
            fb2_b = tCw.tile([P, D], F32, tag="fb2")
            if not skip_gb:
                ffg_b = tCw.tile([P, D], F32, tag="ffg")
                ffb_b = tCw.tile([P, D], F32, tag="ffb")
            else:
                ffg_b = ffb_b = None

            # LN(x2) results, produced in phase B tail, used in phase C
            phBC_cm = tc.tile_pool(name="phBC", bufs=1)
            tBC = phBC_cm.__enter__()
            fabs = [tBC.tile([P, D], BF16, tag=f"fab{j}", name=f"fab{j}")
                    for j in range(NCH)]
            x2b = [tBC.tile([P, D], F32, tag=f"x2b{j}", name=f"x2b{j}")
                   for j in range(NCH)]

            # ---- activations + weights that span phases A+B
            acts_ab_cm = tc.tile_pool(name="actsAB", bufs=1)
            acts_ab = acts_ab_cm.__enter__()
            ctxaT = acts_ab.tile([P, KC, M], F8, tag="ctxaT")
            qT = acts_ab.tile([P, KC, N], BF16, tag="qT")
            kT = acts_ab.tile([P, KC, M], BF16, tag="kT")
            vv = [acts_ab.tile([P, NCH, D], F8, tag=f"vv{h}", name=f"vv{h}")
                  for h in range(H)]
            mw_sb = acts_ab.tile([P, KC * H, D], F8, tag="mw")

            # ---------------- phase A: LN/swish/transpose + projections + v
            with (
                tc.tile_pool(name="phA", bufs=3) as tA,
                tc.tile_pool(name="phA_in", bufs=1) as tAin,
                tc.tile_pool(name="phA_w", bufs=1) as tAw,
                tc.tile_pool(name="pmmA", bufs=4, space="PSUM") as pmm,
                tc.tile_pool(name="ptA", bufs=2, space="PSUM") as ptp,
            ):
                xaT = tAw.tile([P, KC, N], F8, tag="xaT")
                kvwv_sb = tAw.tile([P, KC, D * H], F8, tag="kvwv")
                qw_sb = tAw.tile([P, KC, DH * H], F8, tag="qw")
                kvwk_sb = tAw.tile([P, KC, DH * H], F8, tag="kvwk")
                mbr_b = tAw.tile([P, D], F32, tag="mbr")
                if not skip_gb:
                    qg_b = tAw.tile([P, D], F32, tag="qg")
                    nc.gpsimd.dma_start(qg_b, _bcast_1d(qg_d, D))
                    qb_b = tAw.tile([P, D], F32, tag="qb")
                    nc.gpsimd.dma_start(qb_b, _bcast_1d(qb_d, D))
                    kvg_b = tAw.tile([P, D], F32, tag="kvg")
                    nc.gpsimd.dma_start(kvg_b, _bcast_1d(kvg_d, D))
                    kvb_b = tAw.tile([P, D], F32, tag="kvb")
                    nc.gpsimd.dma_start(kvb_b, _bcast_1d(kvb_d, D))
                    nc.gpsimd.dma_start(ffg_b, _bcast_1d(ffg_d, D))
                    nc.gpsimd.dma_start(ffb_b, _bcast_1d(ffb_d, D))
                else:
                    qg_b = qb_b = kvg_b = kvb_b = None
                nc.gpsimd.dma_start(mbr_b, _bcast_1d(mbr_d, D))
                nc.gpsimd.dma_start(fb2_b, _bcast_1d(fb2_d, D))

                # one DMA queue, priority order: ctx, k/q weights, v weights,
                # x, merge weights, ffn weights -- so nothing starves ctx.
                ctx_t = []
                for j in range(NCH):
                    xt = tAin.tile([P, D], F32, tag=f"lc_in{j}", name="xt")
                    nc.sync.dma_start(xt, ctx_d.ap()[j * P:(j + 1) * P, :])
                    ctx_t.append(xt)
                nc.sync.dma_start(
                    kvwk_sb,
                    kvw_d.ap()[:, 0:DH * H].rearrange("(o p) c -> p o c", p=P),
                )
                nc.sync.dma_start(qw_sb, qw_d.ap().rearrange("(o p) c -> p o c", p=P))
                nc.sync.dma_start(
                    kvwv_sb,
                    kvw_d.ap()[:, DH * H:].rearrange("(o p) c -> p o c", p=P),
                )
                x_t = []
                for j in range(NCH):
                    xt = tAin.tile([P, D], F32, tag=f"lx_in{j}", name="xt2")
                    nc.sync.dma_start(xt, x_d.ap()[j * P:(j + 1) * P, :])
                    x_t.append(xt)
                nc.sync.dma_start(
                    mw_sb, mw_d.ap().rearrange("(o p) c -> p o c", p=P)
                )
                nc.sync.dma_start(
                    fw1_sb, fw1_d.ap().rearrange("(o p) c -> p o c", p=P)
                )
                nc.sync.dma_start(
                    fw2_sb, fw2_d.ap().rearrange("(o p) c -> p o c", p=P)
                )

                mvs_c = tAw.tile([P, 2 * NCH], F32, tag="mvs_c")
                rss_c = tAw.tile([P, NCH], F32, tag="rss_c")
                HB = NCH // 2
                for half in range(2):
                    for j in range(half * HB, (half + 1) * HB):
                        ln_stats(ctx_t[j], mvs_c, rss_c, j, tA, "lc")
                    ln_finish(mvs_c, rss_c, tA, "lc",
                              lo=half * HB, hi=(half + 1) * HB)
                    for j in range(half * HB, (half + 1) * HB):
                        xab = ln_apply(ctx_t[j], mvs_c, rss_c, j, kvg_b,
                                       kvb_b, tA, "lc")
                        transpose_chunk(xab, ctxaT, j, ptp)
                proj_T(kvwk_sb, ctxaT, kT, kvbk_c, pmm)

                # v for all heads (scaled 8x), readouts split Scalar/DVE
                for h in range(H):
                    for i in range(NCH):
                        ps = pmm.tile([P, FD], F32, tag="pmm", name="ps")
                        for g in range(KC // 2):
                            nc.tensor.matmul(
                                ps,
                                lhsT=ctxaT[:, 2 * g:2 * g + 2, i * P:(i + 1) * P],
                                rhs=kvwv_sb[:, 2 * g:2 * g + 2,
                                            h * D:(h + 1) * D],
                                start=(g == 0), stop=(g == KC // 2 - 1),
                                perf_mode=DR,
                            )
                        if h % 2 == 0:
                            nc.scalar.activation(
                                out=vv[h][:, i, :], in_=ps, func=AF.Copy,
                                scale=8.0 / WS,
                            )
                        else:
                            nc.vector.tensor_scalar_mul(
                                out=vv[h][:, i, :], in0=ps, scalar1=8.0 / WS,
                            )

                # x LN (DVE work already overlapped with v) + delta seed
                mvs_x = tAw.tile([P, 2 * NCH], F32, tag="mvs_x")
                rss_x = tAw.tile([P, NCH], F32, tag="rss_x")
                for j in range(NCH):
                    nc.gpsimd.tensor_add(
                        out=delta[j], in0=x_t[j], in1=mbr_b
                    )
                    ln_stats(x_t[j], mvs_x, rss_x, j, tA, "lx")
                ln_finish(mvs_x, rss_x, tA, "lx")
                for j in range(NCH):
                    xab = ln_apply(x_t[j], mvs_x, rss_x, j, qg_b, qb_b,
                                   tA, "lx")
                    transpose_chunk(xab, xaT, j, ptp)
                proj_T(qw_sb, xaT, qT, qbias_c, pmm)

            # ---------------- phase B: per-head-pair attention + merge
            # software-pipelined: S+exp for pair hp while pair hp-1 does
            # den/outT/merge, so PE never waits on the Scalar Exp batch.
            with (
                tc.tile_pool(name="phB", bufs=2) as tB,
                tc.tile_pool(name="phBs", bufs=2) as tBs,
                tc.tile_pool(name="pmmB", bufs=4, space="PSUM") as pmm,
                tc.tile_pool(name="psS", bufs=2, space="PSUM") as psS,
                tc.tile_pool(name="pdenB", bufs=1, space="PSUM") as pden,
            ):
                expS_p = {}

                def emit_S(hp):
                    cc_h = hp
                    expS = [
                        tB.tile([P, NCH, N], F8, tag=f"expS{par}",
                                name=f"expS{hp}_{par}")
                        for par in range(2)
                    ]
                    for i in range(NCH):
                        for ns in range(NS):
                            pse = [None, None]
                            for par in range(2):
                                po = par * DH
                                pse[par] = psS.tile([P, FD], F32, tag="psS",
                                                    name=f"psS{par}")
                                nc.tensor.matmul(
                                    pse[par],
                                    lhsT=kT[po:po + DH, cc_h, i * P:(i + 1) * P],
                                    rhs=qT[po:po + DH, cc_h,
                                           ns * FD:(ns + 1) * FD],
                                    start=True, stop=True,
                                )
                            for par in range(2):
                                nc.scalar.activation(
                                    out=expS[par][:, i, ns * FD:(ns + 1) * FD],
                                    in_=pse[par], func=AF.Exp, scale=SCALE,
                                )
                    expS_p[hp] = expS

                def emit_attn(hp):
                    for par in range(2):
                        h = 2 * hp + par
                        eS = expS_p[hp][par]
                        den_row = tBs.tile([1, N], F32, tag=f"denrow{par}",
                                           name=f"denrow{par}")
                        for ns in range(NS):
                            psd = pden.tile([1, FD], F32, tag="pden",
                                            name="psd")
                            for g in range(NCH // 2):
                                nc.tensor.matmul(
                                    psd,
                                    lhsT=ones8[:, :, 0:1],
                                    rhs=eS[:, 2 * g:2 * g + 2,
                                           ns * FD:(ns + 1) * FD],
                                    start=(g == 0), stop=(g == NCH // 2 - 1),
                                    perf_mode=DR,
                                )
                            nc.vector.tensor_scalar_mul(
                                out=den_row[0:1, ns * FD:(ns + 1) * FD],
                                in0=psd, scalar1=64.0,
                            )
                        recip_col = tBs.tile([P, NCH], F32, tag=f"recip{par}",
                                             name=f"recip{par}")
                        for j in range(NCH):
                            ptd = pden.tile([P, 1], F32, tag="ptd", name="ptd")
                            nc.tensor.matmul(
                                ptd,
                                lhsT=den_row[0:1, j * P:(j + 1) * P],
                                rhs=ones_col[0:1, 0:1],
                                start=True, stop=True,
                            )
                            nc.vector.tensor_copy(
                                out=recip_col[:, j:j + 1], in_=ptd
                            )
                        nc.vector.reciprocal(out=recip_col, in_=recip_col)

                        # outT: ns interleaved inside cc so every LDWEIGHTS
                        # hides under the previous matmul of the same lhsT
                        outT_h = tB.tile([P, KC, N], F8, tag=f"outT{par}",
                                         name=f"outT{par}")
                        for cc in range(KC):
                            pso = [None, None]
                            for g in range(NCH // 2):
                                for ns in range(NS):
                                    if g == 0:
                                        pso[ns] = pmm.tile(
                                            [P, FD], F32, tag="pmm",
                                            name=f"pso{ns}")
                                    nc.tensor.matmul(
                                        pso[ns],
                                        lhsT=vv[h][:, 2 * g:2 * g + 2,
                                                   cc * P:(cc + 1) * P],
                                        rhs=eS[:, 2 * g:2 * g + 2,
                                               ns * FD:(ns + 1) * FD],
                                        start=(g == 0),
                                        stop=(g == NCH // 2 - 1),
                                        perf_mode=DR,
                                    )
                            for ns in range(NS):
                                nc.vector.tensor_scalar_mul(
                                    out=outT_h[:, cc, ns * FD:(ns + 1) * FD],
                                    in0=pso[ns], scalar1=1.0 / 32.0,
                                )

                        for j in range(NCH):
                            ps = pmm.tile([P, FD], F32, tag="pmm", name="ps")
                            for g in range(KC // 2):
                                nc.tensor.matmul(
                                    ps,
                                    lhsT=outT_h[:, 2 * g:2 * g + 2,
                                                j * P:(j + 1) * P],
                                    rhs=mw_sb[:, h * KC + 2 * g:
                                              h * KC + 2 * g + 2, :],
                                    start=(g == 0), stop=(g == KC // 2 - 1),
                                    perf_mode=DR,
                                )
                            dn = tBs.tile([P, FD], F32, tag="dnorm",
                                          name="dn")
                            nc.vector.tensor_scalar_mul(
                                out=dn, in0=ps, scalar1=recip_col[:, j:j + 1]
                            )
                            nc.gpsimd.tensor_add(
                                out=delta[j], in0=delta[j], in1=dn
                            )

                for hp in range(H // 2 + 1):
                    if hp <= H // 2 - 1:
                        emit_S(hp)
                    if hp > 0:
                        emit_attn(hp - 1)
                        del expS_p[hp - 1]

                # x2 LayerNorm emitted here so it overlaps the tail merges
                mvs_f = tBs.tile([P, 2 * NCH], F32, tag="mvs_f")
                rss_f = tBs.tile([P, NCH], F32, tag="rss_f")
                HB = NCH // 2
                for half in range(2):
                    for j in range(half * HB, (half + 1) * HB):
                        nc.gpsimd.tensor_add(out=x2b[j], in0=delta[j],
                                             in1=fb2_b)
                        ln_stats(delta[j], mvs_f, rss_f, j, tBs, "f")
                    ln_finish(mvs_f, rss_f, tBs, "f",
                              lo=half * HB, hi=(half + 1) * HB)
                    for j in range(half * HB, (half + 1) * HB):
                        ln_apply(delta[j], mvs_f, rss_f, j, ffg_b, ffb_b,
                                 tBs, "f", out_t=fabs[j])

            acts_ab_cm.__exit__(None, None, None)

            # ---------------- phase C: FFN + output
            with (
                tc.tile_pool(name="phC", bufs=3) as tC,
                tc.tile_pool(name="phCl", bufs=1) as tCl,
                tc.tile_pool(name="pmmC", bufs=4, space="PSUM") as pmm,
                tc.tile_pool(name="ptC", bufs=2, space="PSUM") as ptp,
            ):
                ffaT = tCl.tile([P, KC, N], F8, tag="ffaT")
                for j in range(NCH):
                    transpose_chunk(fabs[j], ffaT, j, ptp)

                # h1T = swish(ff_w1.T @ ffaT / WS + b1)   [e, n] fp8
                haT = tCl.tile([P, ECH, N], F8, tag="haT")
                for ec in range(ECH):
                    psh = [None, None]
                    for g in range(KC // 2):
                        for ns in range(NS):
                            if g == 0:
                                psh[ns] = pmm.tile([P, FD], F32, tag="pmm",
                                                   name=f"psh{ns}")
                            nc.tensor.matmul(
                                psh[ns],
                                lhsT=fw1_sb[:, 2 * g:2 * g + 2,
                                            ec * P:(ec + 1) * P],
                                rhs=ffaT[:, 2 * g:2 * g + 2,
                                         ns * FD:(ns + 1) * FD],
                                start=(g == 0), stop=(g == KC // 2 - 1),
                                perf_mode=DR,
                            )
                    for ns in range(NS):
                        nc.scalar.activation(
                            out=haT[:, ec, ns * FD:(ns + 1) * FD],
                            in_=psh[ns], func=AF.Silu, scale=1.0 / WS,
                            bias=fb1_c[:, ec:ec + 1],
                        )

                # ff natural [n, c]: out = ps/WS + (x2 + b2)
                for j in range(NCH):
                    ps = pmm.tile([P, FD], F32, tag="pmm", name="ps")
                    for g in range(ECH // 2):
                        nc.tensor.matmul(
                            ps,
                            lhsT=haT[:, 2 * g:2 * g + 2, j * P:(j + 1) * P],
                            rhs=fw2_sb[:, 2 * g:2 * g + 2, :],
                            start=(g == 0), stop=(g == ECH // 2 - 1),
                            perf_mode=DR,
                        )
                    ot = tC.tile([P, D], F32, tag="ot", name="ot")
                    nc.scalar.activation(
                        out=ot, in_=ps, func=AF.Copy, scale=1.0 / WS
                    )
                    nc.vector.tensor_add(out=ot, in0=ot, in1=x2b[j])
                    nc.sync.dma_start(out_d.ap()[j * P:(j + 1) * P, :], ot)

            phBC_cm.__exit__(None, None, None)
            phCw_cm.__exit__(None, None, None)

    return nc


_CACHED = {}


def _get_nc(skip_gb):
    key = f"nc_{skip_gb}"
    if key not in _CACHED:
        _install_compat()
        _CACHED[key] = _build(skip_gb=skip_gb)
    return _CACHED[key]


def kernel(**inputs):
    skip_gb = all(
        np.all(np.asarray(inputs[g]) == 1.0) and np.all(np.asarray(inputs[b]) == 0.0)
        for g, b in (("q_g", "q_b"), ("kv_g", "kv_b"), ("ff_g", "ff_b"))
    )
    nc = _get_nc(skip_gb)
    b = inputs["x"].shape[0]
    assert b == 8
    import ml_dtypes
    fp8_names = {"q_w", "kv_w", "merge_w", "ff_w1", "ff_w2"}
    shared = {}
    for k, v in inputs.items():
        if k in ("x", "context", "merge_b"):
            continue
        v = np.asarray(v)
        if k in fp8_names:
            sv = np.clip(v.astype(np.float64) * WS, -240.0, 240.0)
            shared[k] = np.ascontiguousarray(sv.astype(ml_dtypes.float8_e4m3))
        else:
            shared[k] = np.ascontiguousarray(v.astype(np.float32))
    # fold the v-projection bias through merge_w into the residual seed:
    # attn @ (v + b) @ W == attn @ v @ W + b @ W  (attn rows sum to 1)
    vb = np.asarray(inputs["kv_bias"]).astype(np.float64)[DH * H:]
    mw = np.asarray(inputs["merge_w"]).astype(np.float64)
    mbr = np.asarray(inputs["merge_b"]).astype(np.float64) + vb @ mw
    shared["mbr"] = np.ascontiguousarray(mbr.astype(np.float32))
    in_maps = []
    for i in range(b):
        m = dict(shared)
        m["x"] = np.ascontiguousarray(np.asarray(inputs["x"][i], dtype=np.float32))
        m["context"] = np.ascontiguousarray(
            np.asarray(inputs["context"][i], dtype=np.float32)
        )
        in_maps.append(m)
    res = run_bass_kernel_spmd(nc, in_maps, core_ids=list(range(8)))
    _CACHED["last_results"] = res
    return np.stack([res.results[i]["out"] for i in range(8)])
